# revision 13
# baseline (speedup 1.0000x reference)
"""Trainium2 Bass kernel for nn_DAG_72782515798738.

Math: node j (of M=1280) computes h_j = tanh(b_j + sum_{k<IN+j} W[j,k]*state_k)
over states = [x (IN=1024), h (M)], batch B=8192. Output y = sigmoid(h[HID:]).

Strategy: data-parallel over batch (8 cores x 1024 rows). Per core, the node
recurrence is solved block-by-block (10 blocks of 128 nodes) with a fixed-point
iteration h <- tanh(p + L_diag @ h) in NODE-MAJOR layout ([node, batch] tiles):
L is strictly triangular so the iteration converges superlinearly (error drops
~10x/iteration; ~9 iterations reach fp32 accuracy). Cross-block/input terms p
are accumulated in PSUM by the tensor engine (contraction 128/1024), prefetched
during the previous block's iterations. No transposes, no per-node ops.
"""
import numpy as np

import concourse.bass as bass
import concourse.mybir as mybir
from concourse.tile import TileContext
from concourse.vector_clock import ScopedClock
from concourse.bass_utils import run_bass_kernel_spmd

F32 = mybir.dt.float32
F32R = mybir.dt.float32r   # single-pass fp32 matmul (tf32-class precision, ~3x faster)
AF = mybir.ActivationFunctionType
ALU = mybir.AluOpType

IN, HID, OUT = 1024, 1024, 256
M = HID + OUT          # 1280 computed nodes
B = 8192
NCORES = 8
BC = B // NCORES       # 1024 batch rows per core
K = 128                # node block size
NB = M // K            # 10 blocks
NIT_R = 4              # float32r updates after h0 (truncation ~5e-5 < f32r noise)
NIT_F = 0              # closing full-fp32 updates (needs fp32 operand tiles; off)
HALF = BC // 2         # 512


_wsplit_ctr = [0]


class _TileContextFix(TileContext):
    """This walrus build accepts only ONE embedded sem-wait per instruction;
    split extra waits onto single-wait NOPs, and same for the exit drain."""

    def _add_instruction(self, inst):
        si = getattr(inst, "sync_info", None)
        if si is not None and si.on_wait is not None and len(si.on_wait) > 1:
            waits = list(si.on_wait)
            for w in waits[:-1]:
                _wsplit_ctr[0] += 1
                nop = mybir.InstNoOp(name=f"wsplit_{_wsplit_ctr[0]}", ins=[], outs=[])
                nop.engine = inst.engine
                nop.sync_info = mybir.SyncInfo(on_wait=[w], on_update=[])
                super()._add_instruction(nop)
            si.on_wait = waits[-1:]
        super()._add_instruction(inst)

    def _drain_and_barrier(self, tick_clock, wait_clock):
        nc = self.nc
        probe = nc.sync.nop(nofuse=True, hint="exit_wait_carrier")
        wait_clock.add_sem_waits(probe.ins, ScopedClock({None: tick_clock.global_clock}))
        si = probe.ins.sync_info
        waits = list(si.on_wait) if si is not None and si.on_wait else []
        if len(waits) > 1:
            si.on_wait = waits[:1]
            for w in waits[1:]:
                n2 = nc.sync.nop(nofuse=True, hint="exit_wait_carrier")
                if n2.ins.sync_info is None:
                    n2.ins.sync_info = mybir.SyncInfo(on_wait=[w], on_update=[])
                else:
                    n2.ins.sync_info.on_wait = [w]
        nc.sync.drain()
        nc.all_engine_barrier()
        assert self.sems is not None
        popped = nc._tile_sem_poison_stack.pop()
        assert popped is self._sem_poison
        nc.clear_and_free_semaphores(list(self.sems.allocated().values()))
        nc.all_engine_barrier()


def _build():
    nc = bass.Bass("TRN2", target_bir_lowering=False, debug=False, num_devices=NCORES)

    xT = nc.dram_tensor("xT", [IN, BC], F32R, kind="ExternalInput")
    WxT = nc.dram_tensor("WxT", [IN, M], F32R, kind="ExternalInput")
    LTd = nc.dram_tensor("LT", [M, M], F32R, kind="ExternalInput")
    bd = nc.dram_tensor("bvec", [M, 1], F32, kind="ExternalInput")
    yT = nc.dram_tensor("yT", [OUT, BC], F32, kind="ExternalOutput")

    KT = IN // 128  # 8 contraction tiles for the input matmul

    with _TileContextFix(nc) as tc:
        with (
            tc.tile_pool(name="sb", bufs=1) as sb,
            tc.tile_pool(name="ps", bufs=1, space="PSUM") as ps,
        ):
            # persistent SBUF tiles
            xt = [sb.tile([128, BC], F32R, name=f"xt{t}", tag=f"xt{t}") for t in range(KT)]
            wx = [sb.tile([128, M], F32R, name=f"wx{t}", tag=f"wx{t}") for t in range(KT)]
            # lt[i]: rows = nodes [128i,128i+128), cols = targets [128i, 1280)
            lt = [sb.tile([128, M - 128 * i], F32R, name=f"lt{i}", tag=f"lt{i}") for i in range(NB)]
            hb = [sb.tile([128, BC], F32R, name=f"h{u}", tag=f"h{u}") for u in range(NB)]
            bt = sb.tile([128, NB], F32, name="bt", tag="bt")

            # DMAs spread across engine SWDGE/HWDGE queues, ordered so block 0
            # (then 1, 2, ...) can start as early as possible: first-half xt
            # and the first wx column-block gate p0; the rest streams behind.
            nc.gpsimd.dma_start(out=bt[:], in_=bd.ap().rearrange("(u p) o -> p (u o)", p=128))
            for t in range(KT):
                nc.scalar.dma_start(out=xt[t][:, 0:HALF], in_=xT.ap()[128 * t:128 * (t + 1), 0:HALF])
                nc.gpsimd.dma_start(out=wx[t][:, 0:128], in_=WxT.ap()[128 * t:128 * (t + 1), 0:128])
            for t in range(KT):
                nc.scalar.dma_start(out=xt[t][:, HALF:], in_=xT.ap()[128 * t:128 * (t + 1), HALF:])
                nc.gpsimd.dma_start(out=wx[t][:, 128:256], in_=WxT.ap()[128 * t:128 * (t + 1), 128:256])
            nc.sync.dma_start(out=lt[0][:], in_=LTd.ap()[0:128, 0:])
            for t in range(KT):
                nc.gpsimd.dma_start(out=wx[t][:, 256:], in_=WxT.ap()[128 * t:128 * (t + 1), 256:])
            for i in range(1, NB):
                nc.sync.dma_start(
                    out=lt[i][:], in_=LTd.ap()[128 * i:128 * (i + 1), 128 * i:])

            with (
                tc.tile_pool(name="pp_pool", bufs=2, space="PSUM") as pp_pool,
                tc.tile_pool(name="lh_pool", bufs=2, space="PSUM") as lh_pool,
                tc.tile_pool(name="psb_pool", bufs=2) as psb_pool,
                tc.tile_pool(name="tin_pool", bufs=2, space="PSUM") as tin_pool,
                tc.tile_pool(name="y_pool", bufs=2) as y_pool,
            ):
                def prefetch_input(u, p_ps):
                    """input-matmul contributions to p for block u (start of group)"""
                    for h in range(2):
                        sl = slice(HALF * h, HALF * (h + 1))
                        for t in range(KT):
                            nc.tensor.matmul(
                                p_ps[:, sl],
                                wx[t][:, 128 * u:128 * (u + 1)],
                                xt[t][:, sl],
                                start=(t == 0), stop=False)

                def prefetch_cross(u, p_ps, i, last):
                    """contribution of completed block i (<u) to p of block u"""
                    for h in range(2):
                        sl = slice(HALF * h, HALF * (h + 1))
                        nc.tensor.matmul(
                            p_ps[:, sl],
                            lt[i][:, 128 * (u - i):128 * (u - i + 1)],
                            hb[i][:, sl],
                            start=False, stop=last)

                p_cur = pp_pool.tile([128, BC], F32, name="pp", tag="pp")
                prefetch_input(0, p_cur)

                for u in range(NB):
                    bcol = bt[:, u:u + 1]
                    # h0 = tanh(p + b) straight from PSUM (ACT)
                    for h in range(2):
                        sl = slice(HALF * h, HALF * (h + 1))
                        nc.scalar.activation(hb[u][:, sl], p_cur[:, sl], AF.Tanh, bias=bcol)
                    # p -> SBUF copy (DVE), frees the PSUM accumulator for prefetch
                    p_sb = psb_pool.tile([128, BC], F32, name="psb", tag="psb")
                    for h in range(2):
                        sl = slice(HALF * h, HALF * (h + 1))
                        nc.vector.tensor_copy(p_sb[:, sl], p_cur[:, sl])

                    # fixed-point iterations, two independent batch halves.
                    # NIT_R fp32r rounds converge to ~3e-5, then NIT_F full-fp32
                    # rounds restore fp32-class accuracy.
                    for k in range(NIT_R + NIT_F):
                        for h in range(2):
                            sl = slice(HALF * h, HALF * (h + 1))
                            lh = lh_pool.tile([128, HALF], F32, name="lh", tag="lh")
                            nc.tensor.matmul(
                                lh[:], lt[u][:, 0:128], hb[u][:, sl],
                                start=True, stop=True)
                            tin = tin_pool.tile([128, HALF], F32, name="tin", tag="tin")
                            nc.vector.tensor_tensor(
                                out=tin[:], in0=lh[:], in1=p_sb[:, sl], op=ALU.add)
                            nc.scalar.activation(hb[u][:, sl], tin[:], AF.Tanh, bias=bcol)

                    # prefetch next block's p (fills PE gaps in the iteration
                    # chain): input + cross from blocks <= u; the i=u pair last.
                    if u + 1 < NB:
                        p_nxt = pp_pool.tile([128, BC], F32, name="pp", tag="pp")
                        prefetch_input(u + 1, p_nxt)
                        for i in range(u):
                            prefetch_cross(u + 1, p_nxt, i, last=False)
                        prefetch_cross(u + 1, p_nxt, u, last=True)
                        p_cur = p_nxt

                    # output blocks: y = sigmoid(h), DMA out
                    if u >= NB - 2:
                        yt = y_pool.tile([128, BC], F32, name="y", tag="y")
                        for h in range(2):
                            sl = slice(HALF * h, HALF * (h + 1))
                            nc.scalar.activation(yt[:, sl], hb[u][:, sl], AF.Sigmoid)
                        r0 = 128 * (u - (NB - 2))
                        nc.gpsimd.dma_start(out=yT.ap()[r0:r0 + 128, :], in_=yt[:])
    return nc


def _enable_ldw_opt():
    """Walrus disables its LDWEIGHTS dedup by default; enabling it shaves a
    few percent here (verified correct for this kernel)."""
    import concourse.bass_utils as _bu
    if getattr(_bu.run_command, "_ldw_patched", False):
        return
    _orig = _bu.run_command

    def _patched(argv, **kw):
        try:
            argv = ["--enable-ldw-opt=true" if a == "--enable-ldw-opt=false" else a
                    for a in argv]
        except Exception:
            pass
        return _orig(argv, **kw)

    _patched._ldw_patched = True
    _bu.run_command = _patched


_nc_cache = None


def kernel(x, W, b):
    global _nc_cache
    x = np.asarray(x, dtype=np.float32)
    W = np.asarray(W, dtype=np.float32)
    b = np.asarray(b, dtype=np.float32)

    xT = np.ascontiguousarray(x.T)                       # [IN, B]
    WxT = np.ascontiguousarray(W[:, :IN].T)              # [IN, M]
    LT = np.ascontiguousarray(np.triu(W[:, IN:].T, 1))   # [M, M], LT[i,j]=W[j,IN+i], i<j
    b2 = np.ascontiguousarray(b.reshape(M, 1))

    if _nc_cache is None:
        _enable_ldw_opt()
        _nc_cache = _build()

    in_maps = [
        {"xT": np.ascontiguousarray(xT[:, c * BC:(c + 1) * BC]),
         "WxT": WxT, "LT": LT, "bvec": b2}
        for c in range(NCORES)
    ]
    res = run_bass_kernel_spmd(_nc_cache, in_maps, list(range(NCORES)))
    y = np.concatenate(
        [np.ascontiguousarray(res.results[c]["yT"].T) for c in range(NCORES)], axis=0)
    return y
